# revision 1
# baseline (speedup 1.0000x reference)
"""BoundaryConvLayer Trainium2 kernel: builder + host scheduling.

Sharding: nodes partitioned across 8 cores (12500 each). Each core:
  Phase A: computes the FULL z table (z = x@W_fc^T + b_fc) redundantly
           into its own DRAM (cheaper than collectives at 62 GB/s).
  Phase B: per 128-node dest block: dense mlp/rate/gamma for its shard,
           dma_gather of z[col] rows (edges bucketed by (block, col-chunk),
           int16 chunk-relative indices), segment-sum via one-hot S matmul
           accumulating in PSUM, then the output equation + LayerNorm.
"""
import sys

sys.path.insert(0, "/opt/trn_rl_repo")
import numpy as np
import concourse.bass as bass
import concourse.mybir as mybir
import concourse.tile as tile
from concourse import library_config

F32 = mybir.dt.float32
BF16 = mybir.dt.bfloat16
I16 = mybir.dt.int16
AF = mybir.ActivationFunctionType
ALU = mybir.AluOpType
AX = mybir.AxisListType

EPS = 1e-4
LN_EPS = 1e-5


# ----------------------------------------------------------------- schedule

def build_schedule(edge_index, N, n_cores, d=128):
    """Host-side edge bucketing. Returns a dict with the uniform (cross-core)
    schedule and per-core index/reldest planes."""
    row = np.asarray(edge_index[0], dtype=np.int64)
    col = np.asarray(edge_index[1], dtype=np.int64)
    NS = N // n_cores                      # real nodes per core
    NB = (NS + 127) // 128                 # dest blocks per core
    NTT = (N + 127) // 128                 # full-table tiles
    NTTG = (NTT + 3) // 4                  # phase-A groups of 4 tiles
    NPAD2 = NTTG * 4 * 128                 # padded table rows
    CH = NPAD2 // 4                        # chunk rows (int16-addressable)
    assert CH <= 32768
    SBS = 8                                # blocks per superblock
    sbs = [list(range(s, min(s + SBS, NB))) for s in range(0, NB, SBS)]

    core_of = row // NS
    rrel = row - core_of * NS
    b_of = rrel // 128
    rel_of = rrel - b_of * 128
    k_of = col // CH
    crel_of = col - k_of * CH

    # per (core, b, k) buckets
    counts = np.zeros((n_cores, NB, 4), dtype=np.int64)
    np.add.at(counts, (core_of, b_of, k_of), 1)
    T = np.ceil(counts / 128).astype(np.int64).max(axis=0)   # [NB, 4]
    empty = T.sum(axis=1) == 0
    T[empty, 0] = 1                                          # >=1 tile per block

    # order edges by (core, b, k)
    order = np.lexsort((k_of, b_of, core_of))
    s_core, s_b, s_k = core_of[order], b_of[order], k_of[order]
    s_crel, s_rel = crel_of[order], rel_of[order]
    # bucket start offsets in the sorted stream per (core,b,k)
    flat = (s_core * NB + s_b) * 4 + s_k
    bucket_cnt = np.bincount(flat, minlength=n_cores * NB * 4).reshape(n_cores, NB, 4)
    bucket_off = np.zeros_like(bucket_cnt)
    bucket_off.reshape(-1)[1:] = np.cumsum(bucket_cnt.reshape(-1))[:-1]

    # schedule order: for sb: for k: for b in sb: T[b,k] tiles.
    # Each (sb,k) run is padded to a multiple of CALLQ tiles (dummy tiles
    # gather row 0 of the chunk, rel_dest=-1 so S kills them) so that
    # num_idxs_reg takes few distinct values (register pressure).
    CALLQ = 4
    calls = []            # list of (si, k, [(tau, bi_in_sb, b)...])
    tau = 0
    for si, sb in enumerate(sbs):
        for k in range(4):
            run = []
            for bi, b in enumerate(sb):
                t = int(T[b, k])
                for _ in range(t):
                    run.append((tau, bi, b))
                    tau += 1
            if not run:
                continue
            lb_bi, lb_b = run[-1][1], run[-1][2]
            while len(run) % CALLQ:
                run.append((tau, lb_bi, lb_b))
                tau += 1
            calls.append((si, k, run))
    TOT_TILES = tau
    TOT_SLOTS = TOT_TILES * 128
    MAXNT = max(len(r[2]) for r in calls)

    # per-block total MM count (for PSUM start/stop flags)
    TTb = T.sum(axis=1)

    # per-core planes
    idxp_list, reld_list = [], []
    for c in range(n_cores):
        idx_flat = np.zeros(TOT_SLOTS, dtype=np.int16)
        rel_flat = np.full(TOT_SLOTS, -1.0, dtype=np.float32)
        pos = 0
        for si, sb in enumerate(sbs):
            for k in range(4):
                run_tiles = 0
                for b in sb:
                    t = int(T[b, k])
                    if t == 0:
                        continue
                    n = int(bucket_cnt[c, b, k])
                    o = int(bucket_off[c, b, k])
                    assert n <= t * 128
                    idx_flat[pos:pos + n] = s_crel[o:o + n]
                    rel_flat[pos:pos + n] = s_rel[o:o + n]
                    pos += t * 128
                    run_tiles += t
                if run_tiles:
                    pos += ((-run_tiles) % CALLQ) * 128
        assert pos == TOT_SLOTS
        # pack: slot i -> partition i%16 (replicated x8), col i//16
        idxp = np.tile(idx_flat.reshape(-1, 16).T, (8, 1)).astype(np.int16)
        reld = rel_flat.reshape(-1, 128).T.astype(np.float32)  # [128, TOT_TILES]
        idxp_list.append(np.ascontiguousarray(idxp))
        reld_list.append(np.ascontiguousarray(reld))

    cnt = np.bincount(row, minlength=N).astype(np.float32)
    return dict(NS=NS, NB=NB, NPAD2=NPAD2, CH=CH, NTTG=NTTG, sbs=sbs,
                T=T, TTb=TTb, calls=calls, TOT_TILES=TOT_TILES, MAXNT=MAXNT,
                idxp=idxp_list, reld=reld_list, cnt=cnt, SBS=SBS)


# ----------------------------------------------------------------- post-passes

def patch_library_reloads(nc):
    from concourse import bass_isa
    isa = nc.isa
    e = isa.get_enum("NEURON_ISA_TPB_PSEUDO_OPCODE")
    op = e.NEURON_ISA_TPB_PSEUDO_OPCODE_PSEUDO_LIBRARY_RELOAD_INDEX.value
    for f in nc.m.functions:
        for blk in f.blocks:
            for ins in blk.instructions:
                if type(ins).__name__ == "InstPseudoReloadLibraryIndex" and not ins.instr:
                    instr, fixups = bass_isa.isa_struct(
                        isa, isa.Opcode.NEURON_ISA_TPB_OPCODE_PSEUDO_INST,
                        {"pseudo_opcode": op, "lib_index": ins.lib_index})
                    assert not fixups
                    ins.instr = instr


def split_sync_waits(nc, max_waits=1):
    ctr = 0
    for f in nc.m.functions:
        for blk in f.blocks:
            new_list = []
            for ins in blk.instructions:
                si = ins.sync_info
                if si is not None and si.on_wait and len(si.on_wait) > max_waits:
                    waits = list(si.on_wait)
                    keep = waits[-max_waits:]
                    extra = waits[:-max_waits]
                    for i in range(0, len(extra), max_waits):
                        ctr += 1
                        nop = mybir.InstNoOp(name=f"I-ws-{ctr}", ins=[], outs=[])
                        nop.engine = ins.engine
                        nop.sync_info = mybir.SyncInfo(
                            on_wait=extra[i:i + max_waits], on_update=[])
                        new_list.append(nop)
                    si.on_wait = keep
                new_list.append(ins)
            blk.instructions = new_list
    return ctr


# ----------------------------------------------------------------- bass build

def _emit_ln(nc, pool, x_ap, g_ap, b_ap, out_ap, tagp, d, eps_ap):
    m = pool.tile([128, 1], F32, tag=tagp + "m")
    nc.vector.tensor_reduce(out=m[:], in_=x_ap, axis=AX.X, op=ALU.add)
    nc.scalar.mul(out=m[:], in_=m[:], mul=1.0 / d)
    c = pool.tile([128, d], F32, tag=tagp + "c")
    nc.vector.tensor_scalar(out=c[:], in0=x_ap, scalar1=m[:], scalar2=None,
                            op0=ALU.subtract)
    sq = pool.tile([128, d], F32, tag=tagp + "s")
    v = pool.tile([128, 1], F32, tag=tagp + "v")
    nc.scalar.activation(out=sq[:], in_=c[:], func=AF.Square, accum_out=v[:])
    s = pool.tile([128, 1], F32, tag=tagp + "sd")
    nc.scalar.activation(out=s[:], in_=v[:], func=AF.Ln, scale=1.0 / d,
                         bias=eps_ap)
    r = pool.tile([128, 1], F32, tag=tagp + "r")
    nc.scalar.activation(out=r[:], in_=s[:], func=AF.Exp, scale=-0.5)
    nc.vector.tensor_scalar(out=c[:], in0=c[:], scalar1=r[:], scalar2=None,
                            op0=ALU.mult)
    nc.vector.tensor_tensor(out=c[:], in0=c[:], in1=g_ap, op=ALU.mult)
    nc.vector.tensor_tensor(out=out_ap, in0=c[:], in1=b_ap, op=ALU.add)


def build_bass(sch, n_cores, D=128, DH=256, do_gather=True, do_phase_a=True):
    NS, NB = sch["NS"], sch["NB"]
    NSP = NB * 128
    NPAD2, CH, NTTG = sch["NPAD2"], sch["CH"], sch["NTTG"]
    sbs, T, TTb, calls = sch["sbs"], sch["T"], sch["TTb"], sch["calls"]
    TOT_TILES, MAXNT, SBS = sch["TOT_TILES"], sch["MAXNT"], sch["SBS"]

    nc = bass.Bass("TRN2", target_bir_lowering=False, debug=False,
                   num_devices=n_cores)

    xt_full = nc.declare_dram_parameter("xt_full", [D, NPAD2], BF16, isOutput=False)
    xt_loc = nc.declare_dram_parameter("xt_loc", [D, NSP], F32, isOutput=False)
    wcat = nc.declare_dram_parameter("wcat", [D, D * 2 + DH], F32, isOutput=False)
    wfcb = nc.declare_dram_parameter("wfcb", [D, D], BF16, isOutput=False)
    w2t = nc.declare_dram_parameter("w2t", [DH, D], F32, isOutput=False)
    bfc4 = nc.declare_dram_parameter("bfc4", [128, 512], F32, isOutput=False)
    consts = nc.declare_dram_parameter("consts", [128, D * 7 + DH + 2], F32, isOutput=False)
    # consts cols: bfc(0:D) b2(D:2D) ln1g ln1b ln2g ln2b iota  then b1 (DH)
    idxp_d = nc.declare_dram_parameter("idxp", [128, TOT_TILES * 8], I16, isOutput=False)
    reld_d = nc.declare_dram_parameter("reld", [128, TOT_TILES], F32, isOutput=False)
    cnt_d = nc.declare_dram_parameter("cntp", [128, NB], F32, isOutput=False)
    deg_d = nc.declare_dram_parameter("degp", [128, NB], F32, isOutput=False)
    ident_d = nc.declare_dram_parameter("ident", [128, 128], F32, isOutput=False)
    out_d = nc.declare_dram_parameter("out", [NSP, D], F32, isOutput=True)

    ztab = nc.dram_tensor("ztab", [NPAD2, D], BF16)

    with tile.TileContext(nc) as tc:
        nc.gpsimd.load_library(library_config.mlp)
        with tc.tile_pool(name="cpool", bufs=1) as cp, \
             tc.tile_pool(name="work", bufs=2) as wp, \
             tc.tile_pool(name="psum", bufs=2, space="PSUM") as pp:

            # ---- constants
            wcat_t = cp.tile([D, D * 2 + DH], F32)
            nc.sync.dma_start(out=wcat_t[:], in_=wcat[:])
            wfcb_t = cp.tile([D, D], BF16, tag="wfcb")
            nc.sync.dma_start(out=wfcb_t[:], in_=wfcb[:])
            w2a_t = cp.tile([128, D], F32, tag="w2a")
            nc.sync.dma_start(out=w2a_t[:], in_=w2t[0:128, :])
            w2b_t = cp.tile([128, D], F32, tag="w2b")
            nc.sync.dma_start(out=w2b_t[:], in_=w2t[128:DH, :])
            bfc4_t = cp.tile([128, 512], F32)
            nc.sync.dma_start(out=bfc4_t[:], in_=bfc4[:])
            consts_t = cp.tile([128, D * 7 + DH + 2], F32)
            nc.sync.dma_start(out=consts_t[:], in_=consts[:])
            bfc_t = consts_t[:, 0:D]
            b2_t = consts_t[:, D:2 * D]
            ln1g_t = consts_t[:, 2 * D:3 * D]
            ln1b_t = consts_t[:, 3 * D:4 * D]
            ln2g_t = consts_t[:, 4 * D:5 * D]
            ln2b_t = consts_t[:, 5 * D:6 * D]
            iota_t = consts_t[:, 6 * D:7 * D]
            b1_t = consts_t[:, 7 * D:7 * D + DH]
            eps_t = consts_t[:, 7 * D + DH:7 * D + DH + 1]
            ones_t = consts_t[:, 7 * D + DH + 1:7 * D + DH + 2]
            ident_t = cp.tile([128, 128], F32, tag="ident")
            nc.sync.dma_start(out=ident_t[:], in_=ident_d[:])
            cnt_t = cp.tile([128, NB], F32, tag="cnt")
            nc.sync.dma_start(out=cnt_t[:], in_=cnt_d[:])
            deg_t = cp.tile([128, NB], F32, tag="deg")
            nc.sync.dma_start(out=deg_t[:], in_=deg_d[:])
            reld_t = cp.tile([128, TOT_TILES], F32, tag="reld")
            nc.sync.dma_start(out=reld_t[:], in_=reld_d[:])

            nidx_regs = {}

            # ---- phase A: full z table
            for g in range(NTTG if do_phase_a else 0):
                xa = wp.tile([128, 512], BF16, tag="xa")
                nc.sync.dma_start(out=xa[:], in_=xt_full[:, g * 512:(g + 1) * 512])
                zps = pp.tile([128, 512], F32, tag="ps1")
                for j in range(4):
                    nc.tensor.matmul(out=zps[:, j * 128:(j + 1) * 128],
                                     lhsT=xa[:, j * 128:(j + 1) * 128],
                                     rhs=wfcb_t[:],
                                     start=True, stop=True)
                za = wp.tile([128, 512], BF16, tag="za")
                nc.vector.tensor_add(out=za[:], in0=zps[:], in1=bfc4_t[:])
                for j in range(4):
                    nc.sync.dma_start(
                        out=ztab[g * 512 + j * 128:g * 512 + (j + 1) * 128, :],
                        in_=za[:, j * 128:(j + 1) * 128])

            # ---- phase B
            for si, sb in enumerate(sbs):
                nsb = len(sb)
                z_sb = wp.tile([128, SBS * 128], F32, tag="z_sb", bufs=3)
                rate_sb = wp.tile([128, SBS * 128], F32, tag="rate_sb", bufs=3)
                gam_sb = wp.tile([128, SBS * 128], F32, tag="gam_sb", bufs=3)
                for bi, b in enumerate(sb):
                    sl = slice(bi * 128, (bi + 1) * 128)
                    xb = wp.tile([128, 128], F32, tag="xb")
                    nc.sync.dma_start(out=xb[:], in_=xt_loc[:, b * 128:(b + 1) * 128])
                    ps1 = pp.tile([128, 512], F32, tag="ps1")
                    nc.tensor.matmul(out=ps1[:], lhsT=xb[:], rhs=wcat_t[:],
                                     start=True, stop=True)
                    nc.vector.tensor_add(out=z_sb[:, sl], in0=ps1[:, 0:D], in1=bfc_t)
                    spe = wp.tile([128, 128], F32, tag="spe")
                    nc.scalar.activation(out=spe[:], in_=ps1[:, D:2 * D],
                                         func=AF.Exp)
                    nc.scalar.activation(out=rate_sb[:, sl], in_=spe[:],
                                         func=AF.Ln, bias=ones_t)
                    hp = wp.tile([128, DH], F32, tag="hp")
                    nc.vector.tensor_add(out=hp[:], in0=ps1[:, 2 * D:2 * D + DH],
                                         in1=b1_t)
                    sph = wp.tile([128, DH], F32, tag="sph")
                    nc.scalar.activation(out=sph[:], in_=hp[:], func=AF.Exp)
                    nc.scalar.activation(out=hp[:], in_=sph[:], func=AF.Ln,
                                         bias=ones_t)
                    pst = pp.tile([128, 128], F32, tag="pst")
                    nc.tensor.transpose(out=pst[:], in_=hp[:, 0:128], identity=ident_t[:])
                    ht0 = wp.tile([128, 128], F32, tag="ht0")
                    nc.vector.tensor_copy(out=ht0[:], in_=pst[:])
                    pst2 = pp.tile([128, 128], F32, tag="pst")
                    nc.tensor.transpose(out=pst2[:], in_=hp[:, 128:DH], identity=ident_t[:])
                    ht1 = wp.tile([128, 128], F32, tag="ht1")
                    nc.vector.tensor_copy(out=ht1[:], in_=pst2[:])
                    ps2 = pp.tile([128, 128], F32, tag="ps2", bufs=1)
                    nc.tensor.matmul(out=ps2[:], lhsT=ht0[:], rhs=w2a_t[:],
                                     start=True, stop=False)
                    nc.tensor.matmul(out=ps2[:], lhsT=ht1[:], rhs=w2b_t[:],
                                     start=False, stop=True)
                    g0 = wp.tile([128, 128], F32, tag="g0")
                    nc.vector.tensor_add(out=g0[:], in0=ps2[:], in1=b2_t)
                    _emit_ln(nc, wp, g0[:], ln1g_t, ln1b_t, gam_sb[:, sl], "ln1", D, eps_t)

                # gather + segment-sum
                # PSUM accumulate-bit clearing is per-BANK on start=True, so
                # exactly one start (and one stop) per bank of `agg` per sb.
                agg = pp.tile([128, SBS * 128], F32, tag="agg", bufs=1)
                if not do_gather:
                    nc.vector.memset(agg[:], 0.0)
                sb_calls = [cl for cl in calls if cl[0] == si] if do_gather else []
                mm_bank_seq = []            # bank of each MM in emission order
                for (_, _, run) in sb_calls:
                    for (_, bi, _) in run:
                        mm_bank_seq.append((bi * 128) // 512)
                first_of_bank, last_of_bank = {}, {}
                for i, bk in enumerate(mm_bank_seq):
                    if bk not in first_of_bank:
                        first_of_bank[bk] = i
                    last_of_bank[bk] = i
                mm_i = 0
                for (csi, k, run) in sb_calls:
                    nt = len(run)
                    tau0 = run[0][0]
                    idx_t = wp.tile([128, MAXNT * 8], I16, tag="idx")
                    nc.sync.dma_start(out=idx_t[:, 0:nt * 8],
                                      in_=idxp_d[:, tau0 * 8:(tau0 + nt) * 8])
                    gst = wp.tile([128, MAXNT * 128], BF16, tag="gst", bufs=3)
                    if nt * 128 not in nidx_regs:
                        nidx_regs[nt * 128] = nc.gpsimd.to_reg(nt * 128)
                    nc.gpsimd.dma_gather(
                        out_ap=gst[:, 0:nt * 128].rearrange("p (t e) -> p t e", t=nt),
                        in_ap=ztab[k * CH:(k + 1) * CH, :],
                        idxs_ap=idx_t[:, 0:nt * 8],
                        num_idxs=nt * 128,
                        num_idxs_reg=nidx_regs[nt * 128],
                        elem_size=D,
                        single_packet=(nt * 128 <= 1024))
                    for ti, (tau, bi, b) in enumerate(run):
                        S = wp.tile([128, 128], BF16, tag="S", bufs=4)
                        nc.vector.tensor_scalar(out=S[:], in0=iota_t,
                                                scalar1=reld_t[:, tau:tau + 1],
                                                scalar2=None, op0=ALU.is_equal)
                        bk = mm_bank_seq[mm_i]
                        nc.tensor.matmul(out=agg[:, bi * 128:(bi + 1) * 128],
                                         lhsT=S[:],
                                         rhs=gst[:, ti * 128:(ti + 1) * 128],
                                         start=(first_of_bank[bk] == mm_i),
                                         stop=(last_of_bank[bk] == mm_i),
                                         skip_group_check=True)
                        mm_i += 1

                # finalize
                for bi, b in enumerate(sb):
                    sl = slice(bi * 128, (bi + 1) * 128)
                    t1 = wp.tile([128, 128], F32, tag="fin1")
                    nc.vector.tensor_scalar(out=t1[:], in0=z_sb[:, sl],
                                            scalar1=cnt_t[:, b:b + 1],
                                            scalar2=None, op0=ALU.mult)
                    nc.vector.tensor_add(out=t1[:], in0=t1[:], in1=agg[:, sl])
                    num = wp.tile([128, 128], F32, tag="fin2")
                    nc.vector.tensor_tensor(out=num[:], in0=rate_sb[:, sl],
                                            in1=t1[:], op=ALU.mult)
                    nc.vector.tensor_add(out=num[:], in0=num[:], in1=gam_sb[:, sl])
                    den = wp.tile([128, 128], F32, tag="fin3")
                    nc.vector.tensor_scalar(out=den[:], in0=rate_sb[:, sl],
                                            scalar1=deg_t[:, b:b + 1],
                                            scalar2=1.0 + EPS,
                                            op0=ALU.mult, op1=ALU.add)
                    nc.scalar.activation(out=den[:], in_=den[:], func=AF.Ln)
                    nc.scalar.activation(out=den[:], in_=den[:], func=AF.Exp,
                                         scale=-1.0)
                    nc.vector.tensor_tensor(out=num[:], in0=num[:], in1=den[:],
                                            op=ALU.mult)
                    nc.vector.tensor_tensor(out=num[:], in0=num[:], in1=z_sb[:, sl],
                                            op=ALU.subtract)
                    of = wp.tile([128, 128], F32, tag="of")
                    _emit_ln(nc, wp, num[:], ln2g_t, ln2b_t, of[:], "ln2", D, eps_t)
                    nc.sync.dma_start(out=out_d[b * 128:(b + 1) * 128, :], in_=of[:])

    return nc


def postprocess_for_hw(nc):
    """Must run after build_bass and before NEFF compile (not before CoreSim)."""
    patch_library_reloads(nc)
    split_sync_waits(nc, max_waits=1)


# ----------------------------------------------------------------- host prep

def prepare_core_inputs(x, degree, W_fc, b_fc, W_rate, W1, b1, W2, b2,
                        ln1_g, ln1_b, ln2_g, ln2_b, sch, n_cores, D=128, DH=256):
    """Returns (shared_inputs dict, per_core list of dicts)."""
    N = x.shape[0]
    NS, NB, NPAD2 = sch["NS"], sch["NB"], sch["NPAD2"]
    NSP = NB * 128
    import ml_dtypes
    xt_full = np.zeros((D, NPAD2), dtype=ml_dtypes.bfloat16)
    xt_full[:, :N] = x.T.astype(ml_dtypes.bfloat16)
    wcat = np.concatenate([W_fc.T, W_rate.T, W1.T], axis=1).astype(np.float32)
    w2t = np.ascontiguousarray(W2.T.astype(np.float32))
    bfc4 = np.tile(b_fc.astype(np.float32), (128, 4))
    iota = np.tile(np.arange(D, dtype=np.float32), (128, 1))
    consts = np.concatenate([
        np.tile(b_fc.astype(np.float32), (128, 1)),
        np.tile(b2.astype(np.float32), (128, 1)),
        np.tile(ln1_g.astype(np.float32), (128, 1)),
        np.tile(ln1_b.astype(np.float32), (128, 1)),
        np.tile(ln2_g.astype(np.float32), (128, 1)),
        np.tile(ln2_b.astype(np.float32), (128, 1)),
        iota,
        np.tile(b1.astype(np.float32), (128, 1)),
        np.full((128, 1), LN_EPS, dtype=np.float32),
        np.full((128, 1), 1.0, dtype=np.float32),
    ], axis=1)
    ident = np.eye(128, dtype=np.float32)
    wfcb = np.ascontiguousarray(W_fc.T.astype(ml_dtypes.bfloat16))
    shared = dict(xt_full=np.ascontiguousarray(xt_full), wcat=wcat, w2t=w2t,
                  bfc4=bfc4, consts=np.ascontiguousarray(consts), ident=ident,
                  wfcb=wfcb)

    cnt = sch["cnt"]
    per_core = []
    for c in range(n_cores):
        xt_loc = np.zeros((D, NSP), dtype=np.float32)
        xt_loc[:, :NS] = x[c * NS:(c + 1) * NS].T
        cntp = np.zeros((128, NB), dtype=np.float32)
        degp = np.zeros((128, NB), dtype=np.float32)
        cseg = np.zeros(NSP, dtype=np.float32)
        cseg[:NS] = cnt[c * NS:(c + 1) * NS]
        dseg = np.zeros(NSP, dtype=np.float32)
        dseg[:NS] = degree[c * NS:(c + 1) * NS]
        cntp[:, :] = cseg.reshape(NB, 128).T
        degp[:, :] = dseg.reshape(NB, 128).T
        per_core.append(dict(xt_loc=np.ascontiguousarray(xt_loc),
                             idxp=sch["idxp"][c], reld=sch["reld"][c],
                             cntp=np.ascontiguousarray(cntp),
                             degp=np.ascontiguousarray(degp)))
    return shared, per_core


# ----------------------------------------------------------------- numpy ref

def numpy_reference(x, edge_index, degree, W_fc, b_fc, W_rate, W1, b1, W2, b2,
                    ln1_g, ln1_b, ln2_g, ln2_b):
    def ln(v, g, b):
        m = v.mean(-1, keepdims=True)
        var = ((v - m) ** 2).mean(-1, keepdims=True)
        return (v - m) / np.sqrt(var + LN_EPS) * g + b

    def softplus(v):
        return np.log1p(np.exp(-np.abs(v))) + np.maximum(v, 0)

    rate = softplus(x @ W_rate.T)
    h = softplus(x @ W1.T + b1)
    gamma = ln(h @ W2.T + b2, ln1_g, ln1_b)
    z = x @ W_fc.T + b_fc
    row, col = edge_index[0], edge_index[1]
    msg = z[row] + z[col]
    agg = np.zeros_like(z)
    np.add.at(agg, row, msg)
    out = (rate * agg + gamma) / (1.0 + rate * degree[:, None] + EPS) - z
    return ln(out, ln2_g, ln2_b)


# ----------------------------------------------------------------- runner

N_CORES = 8
_SHARED_NAMES = ("xt_full", "wcat", "w2t", "bfc4", "consts", "ident", "wfcb")


def make_runner(nc, shared, per_core, n_cores=N_CORES):
    """Compile nc via PJRT/axon and return (fn, dev_args, out_info).

    Shared inputs are replicated (one upload), per-core inputs sharded on
    axis 0. Output buffers are passed as (unread) operands so repeated calls
    need no fresh allocations. Call fn(*dev_args) -> tuple of out arrays.
    """
    import jax
    from jax.sharding import Mesh, PartitionSpec, NamedSharding
    from jax.experimental.shard_map import shard_map
    from concourse import bass2jax

    bass2jax.install_neuronx_cc_hook()

    in_names, out_names, out_avals, zero_outs = [], [], [], []
    partition_name = (nc.partition_id_tensor.name
                      if nc.partition_id_tensor else None)
    for alloc in nc.m.functions[0].allocations:
        if not isinstance(alloc, mybir.MemoryLocationSet):
            continue
        name = alloc.memorylocations[0].name
        if alloc.kind == "ExternalInput":
            if name != partition_name:
                in_names.append(name)
        elif alloc.kind == "ExternalOutput":
            shape = tuple(alloc.tensor_shape)
            dtype = mybir.dt.np(alloc.dtype)
            out_names.append(name)
            out_avals.append(jax.core.ShapedArray(shape, dtype))
            zero_outs.append(np.zeros(shape, dtype))
    n_params = len(in_names)
    all_in = list(in_names) + list(out_names)
    if partition_name is not None:
        all_in.append(partition_name)

    def _body(*args):
        operands = list(args)
        if partition_name is not None:
            operands.append(bass2jax.partition_id_tensor())
        outs = bass2jax._bass_exec_p.bind(
            *operands,
            out_avals=tuple(out_avals),
            in_names=tuple(all_in),
            out_names=tuple(out_names),
            lowering_input_output_aliases=(),
            sim_require_finite=True,
            sim_require_nnan=True,
            nc=nc)
        return tuple(outs)

    devices = jax.devices()[:n_cores]
    mesh = Mesh(np.asarray(devices), ("core",))
    specs = []
    host_args = []
    for name in in_names:
        if name in _SHARED_NAMES:
            specs.append(PartitionSpec())
            host_args.append(np.asarray(shared[name]))
        else:
            specs.append(PartitionSpec("core"))
            host_args.append(np.concatenate(
                [np.asarray(per_core[c][name]) for c in range(n_cores)], axis=0))
    for z in zero_outs:
        specs.append(PartitionSpec("core"))
        host_args.append(np.zeros((n_cores * z.shape[0], *z.shape[1:]), z.dtype))
    out_specs = (PartitionSpec("core"),) * len(out_names)

    def _chain(rep):
        def body(*args):
            ins = list(args[:n_params])
            outs = tuple(args[n_params:])
            for _ in range(rep):
                outs = _body(*ins, *outs)
            return outs
        return body

    fn = jax.jit(shard_map(_chain(1), mesh=mesh, in_specs=tuple(specs),
                           out_specs=out_specs, check_rep=False),
                 keep_unused=True)
    fn_rep = None  # multi-call chains unsupported by neuronx_cc_hook
    dev_args = [jax.device_put(a, NamedSharding(mesh, s))
                for a, s in zip(host_args, specs)]
    out_shapes = [tuple(a.shape) for a in out_avals]
    return fn, dev_args, (out_names, out_shapes, fn_rep)


def _prepare_all(inputs, n_cores=N_CORES):
    x = np.asarray(inputs["x"], dtype=np.float32)
    N = x.shape[0]
    sch = build_schedule(np.asarray(inputs["edge_index"]), N, n_cores)
    nc = build_bass(sch, n_cores)
    postprocess_for_hw(nc)
    shared, per_core = prepare_core_inputs(
        x, np.asarray(inputs["degree"], dtype=np.float32),
        np.asarray(inputs["W_fc"]), np.asarray(inputs["b_fc"]),
        np.asarray(inputs["W_rate"]), np.asarray(inputs["W1"]),
        np.asarray(inputs["b1"]), np.asarray(inputs["W2"]),
        np.asarray(inputs["b2"]), np.asarray(inputs["ln1_g"]),
        np.asarray(inputs["ln1_b"]), np.asarray(inputs["ln2_g"]),
        np.asarray(inputs["ln2_b"]), sch, n_cores)
    return sch, nc, shared, per_core


def run_kernel(inputs, n_cores=N_CORES, time_reps=0):
    """Returns (out [N, D] fp32, exec_ns or None)."""
    import jax, time as _time
    sch, nc, shared, per_core = _prepare_all(inputs, n_cores)
    fn, dev_args, (out_names, out_shapes, fn_rep) = make_runner(
        nc, shared, per_core, n_cores)
    outs = fn(*dev_args)
    jax.block_until_ready(outs)
    exec_ns = None
    if time_reps:
        # wall time of a dispatch; dominated by ~30-75 ms axon RPC overhead,
        # so this is an upper bound on device time.
        ts = []
        for _ in range(time_reps):
            t0 = _time.perf_counter()
            o1 = fn(*dev_args); jax.block_until_ready(o1)
            ts.append(_time.perf_counter() - t0)
        exec_ns = int(min(ts) * 1e9)
    oi = out_names.index("out")
    N = np.asarray(inputs["x"]).shape[0]
    NS, rows = sch["NS"], out_shapes[oi][0]
    full = np.asarray(outs[oi]).reshape(n_cores, rows, -1)
    out = np.concatenate([full[c, :NS] for c in range(n_cores)], axis=0)[:N]
    return np.ascontiguousarray(out.astype(np.float32)), exec_ns


def kernel(**inputs):
    out, _ = run_kernel(inputs)
    return out



# revision 5
# speedup vs baseline: 1.1876x; 1.1876x over previous
"""BoundaryConvLayer Trainium2 kernel: builder + host scheduling.

Sharding: nodes partitioned across 8 cores (12500 each). Each core:
  Phase A: computes the FULL z table (zn = x@W_fc^T, NO bias) redundantly
           into its own DRAM. Groups of 1024 rows; node order inside each
           group is permuted host-side (node = g*1024 + p*8 + j) so the
           ztab store is a single DMA with 2KB-contiguous runs/partition.
  Phase B: per 128-node dest block: dense mlp/rate/gamma for its shard,
           dma_gather of zn[col] rows (edges bucketed by (block, col-chunk),
           int16 chunk-relative indices), segment-sum via one-hot S matmul
           accumulating in PSUM, then the output equation + LayerNorm.
           S tiles for a whole gather run are built in ONE DVE op via
           stride-0 broadcast APs. hT for the W2 matmul is produced
           directly by W1^T matmuls (no PE transpose). Output rows are
           written superblock-wide in a permuted order; the host undoes
           the permutation after the run.
"""
import sys

sys.path.insert(0, "/opt/trn_rl_repo")
import numpy as np
import concourse.bass as bass
import concourse.mybir as mybir
import concourse.tile as tile
from concourse import library_config

F32 = mybir.dt.float32
BF16 = mybir.dt.bfloat16
I16 = mybir.dt.int16
AF = mybir.ActivationFunctionType
ALU = mybir.AluOpType
AX = mybir.AxisListType

EPS = 1e-4
LN_EPS = 1e-5


# ----------------------------------------------------------------- schedule

def build_schedule(edge_index, N, n_cores, d=128):
    """Host-side edge bucketing. Returns a dict with the uniform (cross-core)
    schedule and per-core index/reldest planes."""
    import ml_dtypes
    row = np.asarray(edge_index[0], dtype=np.int64)
    col = np.asarray(edge_index[1], dtype=np.int64)
    NS = N // n_cores                      # real nodes per core
    NB = (NS + 127) // 128                 # dest blocks per core
    NTT = (N + 127) // 128                 # full-table tiles
    NTTG = (NTT + 3) // 4                  # groups of 4 tiles
    NPAD2 = NTTG * 4 * 128                 # padded table rows
    CH = NPAD2 // 4                        # chunk rows (int16-addressable)
    assert CH <= 32768
    assert NPAD2 % 1024 == 0
    SBS = 8                                # blocks per superblock
    sbs = [list(range(s, min(s + SBS, NB))) for s in range(0, NB, SBS)]

    core_of = row // NS
    rrel = row - core_of * NS
    b_of = rrel // 128
    rel_of = rrel - b_of * 128
    k_of = col // CH
    crel_of = col - k_of * CH

    # per (core, b, k) buckets
    counts = np.zeros((n_cores, NB, 4), dtype=np.int64)
    np.add.at(counts, (core_of, b_of, k_of), 1)
    T = np.ceil(counts / 128).astype(np.int64).max(axis=0)   # [NB, 4]
    empty = T.sum(axis=1) == 0
    T[empty, 0] = 1                                          # >=1 tile per block

    # order edges by (core, b, k)
    order = np.lexsort((k_of, b_of, core_of))
    s_core, s_b, s_k = core_of[order], b_of[order], k_of[order]
    s_crel, s_rel = crel_of[order], rel_of[order]
    # bucket start offsets in the sorted stream per (core,b,k)
    flat = (s_core * NB + s_b) * 4 + s_k
    bucket_cnt = np.bincount(flat, minlength=n_cores * NB * 4).reshape(n_cores, NB, 4)
    bucket_off = np.zeros_like(bucket_cnt)
    bucket_off.reshape(-1)[1:] = np.cumsum(bucket_cnt.reshape(-1))[:-1]

    # schedule order: for sb: for k: for b in sb: T[b,k] tiles.
    # Each (sb,k) run is padded to a multiple of CALLQ tiles (dummy tiles
    # gather row 0 of the chunk, rel_dest=-1 so S kills them) so that
    # num_idxs_reg takes few distinct values (register pressure).
    CALLQ = 4
    calls = []            # list of (si, k, [(tau, bi_in_sb, b)...])
    tau = 0
    for si, sb in enumerate(sbs):
        for k in range(4):
            run = []
            for bi, b in enumerate(sb):
                t = int(T[b, k])
                for _ in range(t):
                    run.append((tau, bi, b))
                    tau += 1
            if not run:
                continue
            lb_bi, lb_b = run[-1][1], run[-1][2]
            while len(run) % CALLQ:
                run.append((tau, lb_bi, lb_b))
                tau += 1
            calls.append((si, k, run))
    TOT_TILES = tau
    TOT_SLOTS = TOT_TILES * 128
    MAXNT = max(len(r[2]) for r in calls)

    # per-block total MM count (for PSUM start/stop flags)
    TTb = T.sum(axis=1)

    # per-core planes
    idxp_list, reld_list = [], []
    for c in range(n_cores):
        idx_flat = np.zeros(TOT_SLOTS, dtype=np.int16)
        rel_flat = np.full(TOT_SLOTS, -1.0, dtype=np.float32)
        pos = 0
        for si, sb in enumerate(sbs):
            for k in range(4):
                run_tiles = 0
                for b in sb:
                    t = int(T[b, k])
                    if t == 0:
                        continue
                    n = int(bucket_cnt[c, b, k])
                    o = int(bucket_off[c, b, k])
                    assert n <= t * 128
                    idx_flat[pos:pos + n] = s_crel[o:o + n]
                    rel_flat[pos:pos + n] = s_rel[o:o + n]
                    pos += t * 128
                    run_tiles += t
                if run_tiles:
                    pos += ((-run_tiles) % CALLQ) * 128
        assert pos == TOT_SLOTS
        # pack: slot i -> partition i%16 (replicated x8), col i//16
        idxp = np.tile(idx_flat.reshape(-1, 16).T, (8, 1)).astype(np.int16)
        reld = rel_flat.reshape(-1, 128).T.astype(ml_dtypes.bfloat16)
        idxp_list.append(np.ascontiguousarray(idxp))
        reld_list.append(np.ascontiguousarray(reld))

    cnt = np.bincount(row, minlength=N).astype(np.float32)
    return dict(NS=NS, NB=NB, NPAD2=NPAD2, CH=CH, sbs=sbs,
                T=T, TTb=TTb, calls=calls, TOT_TILES=TOT_TILES, MAXNT=MAXNT,
                idxp=idxp_list, reld=reld_list, cnt=cnt, SBS=SBS)


# ----------------------------------------------------------------- post-passes

def patch_library_reloads(nc):
    from concourse import bass_isa
    isa = nc.isa
    e = isa.get_enum("NEURON_ISA_TPB_PSEUDO_OPCODE")
    op = e.NEURON_ISA_TPB_PSEUDO_OPCODE_PSEUDO_LIBRARY_RELOAD_INDEX.value
    for f in nc.m.functions:
        for blk in f.blocks:
            for ins in blk.instructions:
                if type(ins).__name__ == "InstPseudoReloadLibraryIndex" and not ins.instr:
                    instr, fixups = bass_isa.isa_struct(
                        isa, isa.Opcode.NEURON_ISA_TPB_OPCODE_PSEUDO_INST,
                        {"pseudo_opcode": op, "lib_index": ins.lib_index})
                    assert not fixups
                    ins.instr = instr


def split_sync_waits(nc, max_waits=1):
    ctr = 0
    for f in nc.m.functions:
        for blk in f.blocks:
            new_list = []
            for ins in blk.instructions:
                si = ins.sync_info
                if si is not None and si.on_wait and len(si.on_wait) > max_waits:
                    waits = list(si.on_wait)
                    keep = waits[-max_waits:]
                    extra = waits[:-max_waits]
                    for i in range(0, len(extra), max_waits):
                        ctr += 1
                        nop = mybir.InstNoOp(name=f"I-ws-{ctr}", ins=[], outs=[])
                        nop.engine = ins.engine
                        nop.sync_info = mybir.SyncInfo(
                            on_wait=extra[i:i + max_waits], on_update=[])
                        new_list.append(nop)
                    si.on_wait = keep
                new_list.append(ins)
            blk.instructions = new_list
    return ctr


# ----------------------------------------------------------------- bass build

def _bcast_ap(ap, dims):
    """Build an AP on the same tensor/offset with an explicit layout.

    dims: list of [step, nelem]; step 0 broadcasts."""
    return bass.AP(ap.tensor, ap.offset, [list(d) for d in dims])


def _emit_ln(nc, pool, x_ap, g_ap, b_ap, out_ap, tagp, d, eps_ap):
    m = pool.tile([128, 1], F32, tag=tagp + "m")
    nc.vector.tensor_reduce(out=m[:], in_=x_ap, axis=AX.X, op=ALU.add)
    nc.scalar.mul(out=m[:], in_=m[:], mul=1.0 / d)
    c = pool.tile([128, d], F32, tag=tagp + "c")
    nc.vector.tensor_scalar(out=c[:], in0=x_ap, scalar1=m[:], scalar2=None,
                            op0=ALU.subtract)
    sq = pool.tile([128, d], F32, tag=tagp + "s")
    v = pool.tile([128, 1], F32, tag=tagp + "v")
    nc.scalar.activation(out=sq[:], in_=c[:], func=AF.Square, accum_out=v[:])
    s = pool.tile([128, 1], F32, tag=tagp + "sd")
    nc.scalar.activation(out=s[:], in_=v[:], func=AF.Ln, scale=1.0 / d,
                         bias=eps_ap)
    r = pool.tile([128, 1], F32, tag=tagp + "r")
    nc.scalar.activation(out=r[:], in_=s[:], func=AF.Exp, scale=-0.5)
    nc.vector.scalar_tensor_tensor(out=c[:], in0=c[:], scalar=r[:], in1=g_ap,
                                   op0=ALU.mult, op1=ALU.mult)
    nc.vector.tensor_tensor(out=out_ap, in0=c[:], in1=b_ap, op=ALU.add)


def build_bass(sch, n_cores, D=128, DH=256, do_gather=True, do_phase_a=True):
    NS, NB = sch["NS"], sch["NB"]
    NSP = NB * 128
    NPAD2, CH = sch["NPAD2"], sch["CH"]
    sbs, T, TTb, calls = sch["sbs"], sch["T"], sch["TTb"], sch["calls"]
    TOT_TILES, MAXNT, SBS = sch["TOT_TILES"], sch["MAXNT"], sch["SBS"]
    NG = NPAD2 // 1024                     # phase-A groups of 1024 rows

    nc = bass.Bass("TRN2", target_bir_lowering=False, debug=False,
                   num_devices=n_cores)

    xt_full = nc.declare_dram_parameter("xt_full", [D, NPAD2], BF16, isOutput=False)
    xt_loc = nc.declare_dram_parameter("xt_loc", [D, NSP], F32, isOutput=False)
    wcat = nc.declare_dram_parameter("wcat", [D, D * 2], F32, isOutput=False)
    w1t = nc.declare_dram_parameter("w1t", [D, DH], F32, isOutput=False)
    wfcb = nc.declare_dram_parameter("wfcb", [D, D], BF16, isOutput=False)
    w2t = nc.declare_dram_parameter("w2t", [DH, D], F32, isOutput=False)
    consts = nc.declare_dram_parameter("consts", [128, D * 6 + 4], F32, isOutput=False)
    # consts cols: bfc(0:D) b2 ln1g ln1b ln2g ln2b then eps, one, b1p(2)
    iotab = nc.declare_dram_parameter("iotab", [128, 128], BF16, isOutput=False)
    idxp_d = nc.declare_dram_parameter("idxp", [128, TOT_TILES * 8], I16, isOutput=False)
    reld_d = nc.declare_dram_parameter("reld", [128, TOT_TILES], BF16, isOutput=False)
    cnt_d = nc.declare_dram_parameter("cntp", [128, NB], F32, isOutput=False)
    deg_d = nc.declare_dram_parameter("degp", [128, NB], F32, isOutput=False)
    out_d = nc.declare_dram_parameter("out", [NSP, D], F32, isOutput=True)

    ztab = nc.dram_tensor("ztab", [NPAD2, D], BF16)

    with tile.TileContext(nc) as tc:
        nc.gpsimd.load_library(library_config.mlp)
        with tc.tile_pool(name="cpool", bufs=1) as cp, \
             tc.tile_pool(name="work", bufs=2) as wp, \
             tc.tile_pool(name="psum", bufs=2, space="PSUM") as pp:

            # ---- constants
            wcat_t = cp.tile([D, D * 2], F32)
            nc.sync.dma_start(out=wcat_t[:], in_=wcat[:])
            w1t_t = cp.tile([D, DH], F32, tag="w1t")
            nc.sync.dma_start(out=w1t_t[:], in_=w1t[:])
            wfcb_t = cp.tile([D, D], BF16, tag="wfcb")
            nc.sync.dma_start(out=wfcb_t[:], in_=wfcb[:])
            w2a_t = cp.tile([128, D], F32, tag="w2a")
            nc.sync.dma_start(out=w2a_t[:], in_=w2t[0:128, :])
            w2b_t = cp.tile([128, D], F32, tag="w2b")
            nc.sync.dma_start(out=w2b_t[:], in_=w2t[128:DH, :])
            consts_t = cp.tile([128, D * 6 + 4], F32)
            nc.sync.dma_start(out=consts_t[:], in_=consts[:])
            bfc_t = consts_t[:, 0:D]
            b2_t = consts_t[:, D:2 * D]
            ln1g_t = consts_t[:, 2 * D:3 * D]
            ln1b_t = consts_t[:, 3 * D:4 * D]
            ln2g_t = consts_t[:, 4 * D:5 * D]
            ln2b_t = consts_t[:, 5 * D:6 * D]
            eps_t = consts_t[:, 6 * D:6 * D + 1]
            ones_t = consts_t[:, 6 * D + 1:6 * D + 2]
            b1p_t = consts_t[:, 6 * D + 2:6 * D + 4]
            iota_t = cp.tile([128, 128], BF16, tag="iota")
            nc.sync.dma_start(out=iota_t[:], in_=iotab[:])
            cnt_t = cp.tile([128, NB], F32, tag="cnt")
            nc.sync.dma_start(out=cnt_t[:], in_=cnt_d[:])
            deg_t = cp.tile([128, NB], F32, tag="deg")
            nc.sync.dma_start(out=deg_t[:], in_=deg_d[:])
            reld_t = cp.tile([128, TOT_TILES], BF16, tag="reld")
            nc.sync.dma_start(out=reld_t[:], in_=reld_d[:])
            idx_all = cp.tile([128, TOT_TILES * 8], I16, tag="idxall")
            nc.sync.dma_start(out=idx_all[:], in_=idxp_d[:])

            nidx_regs = {}

            # ---- phase A: full zn table (no bias), permuted node order so
            # each group's store is one DMA with 2KB runs per partition.
            for g in range(NG if do_phase_a else 0):
                xa = wp.tile([128, 1024], BF16, tag="xa")
                nc.sync.dma_start(out=xa[:], in_=xt_full[:, g * 1024:(g + 1) * 1024])
                za = wp.tile([128, 1024], BF16, tag="za")
                for h in range(2):
                    ps = pp.tile([128, 512], F32, tag="psA")
                    for jj in range(4):
                        j = h * 4 + jj
                        nc.tensor.matmul(out=ps[:, jj * 128:(jj + 1) * 128],
                                         lhsT=xa[:, j * 128:(j + 1) * 128],
                                         rhs=wfcb_t[:],
                                         start=True, stop=True)
                    nc.scalar.activation(out=za[:, h * 512:(h + 1) * 512],
                                         in_=ps[:], func=AF.Copy)
                nc.sync.dma_start(
                    out=ztab[g * 1024:(g + 1) * 1024, :].rearrange(
                        "(p j) c -> p (j c)", p=128),
                    in_=za[:])

            # ---- phase B
            for si, sb in enumerate(sbs):
                nsb = len(sb)
                zn_sb = wp.tile([128, SBS * 128], F32, tag="zn_sb", bufs=3)
                rate_sb = wp.tile([128, SBS * 128], F32, tag="rate_sb", bufs=3)
                gam_sb = wp.tile([128, SBS * 128], F32, tag="gam_sb", bufs=3)
                out_sb = wp.tile([128, SBS * 128], F32, tag="out_sb", bufs=2)
                xb_sb = wp.tile([128, SBS * 128], F32, tag="xb_sb")
                nc.sync.dma_start(
                    out=xb_sb[:, 0:nsb * 128],
                    in_=xt_loc[:, sb[0] * 128:sb[0] * 128 + nsb * 128])
                for bi, b in enumerate(sb):
                    sl = slice(bi * 128, (bi + 1) * 128)
                    ps1 = pp.tile([128, 256], F32, tag="ps1")
                    nc.tensor.matmul(out=ps1[:], lhsT=xb_sb[:, sl], rhs=wcat_t[:],
                                     start=True, stop=True)
                    nc.scalar.activation(out=zn_sb[:, sl], in_=ps1[:, 0:D],
                                         func=AF.Copy)
                    spe = wp.tile([128, 128], F32, tag="spe")
                    nc.scalar.activation(out=spe[:], in_=ps1[:, D:2 * D],
                                         func=AF.Exp)
                    nc.scalar.activation(out=rate_sb[:, sl], in_=spe[:],
                                         func=AF.Ln, bias=ones_t)
                    psh = pp.tile([128, 256], F32, tag="psh", bufs=1)
                    nc.tensor.matmul(out=psh[:, 0:128], lhsT=w1t_t[:, 0:128],
                                     rhs=xb_sb[:, sl], start=True, stop=True)
                    nc.tensor.matmul(out=psh[:, 128:256], lhsT=w1t_t[:, 128:256],
                                     rhs=xb_sb[:, sl], start=True, stop=True)
                    hT = wp.tile([128, 256], F32, tag="hT")
                    he = wp.tile([128, 256], F32, tag="he")
                    nc.scalar.activation(out=he[:, 0:128], in_=psh[:, 0:128],
                                         func=AF.Exp, bias=b1p_t[:, 0:1])
                    nc.scalar.activation(out=he[:, 128:256], in_=psh[:, 128:256],
                                         func=AF.Exp, bias=b1p_t[:, 1:2])
                    nc.scalar.activation(out=hT[:], in_=he[:], func=AF.Ln,
                                         bias=ones_t)
                    ps2 = pp.tile([128, 128], F32, tag="ps2", bufs=1)
                    nc.tensor.matmul(out=ps2[:], lhsT=hT[:, 0:128], rhs=w2a_t[:],
                                     start=True, stop=False)
                    nc.tensor.matmul(out=ps2[:], lhsT=hT[:, 128:256], rhs=w2b_t[:],
                                     start=False, stop=True)
                    g0 = wp.tile([128, 128], F32, tag="g0")
                    nc.vector.tensor_add(out=g0[:], in0=ps2[:], in1=b2_t)
                    _emit_ln(nc, wp, g0[:], ln1g_t, ln1b_t, gam_sb[:, sl], "ln1", D, eps_t)

                # gather + segment-sum
                # PSUM accumulate-bit clearing is per-BANK on start=True, so
                # exactly one start (and one stop) per bank of `agg` per sb.
                agg = pp.tile([128, SBS * 128], F32, tag="agg", bufs=1)
                if not do_gather:
                    nc.vector.memset(agg[:], 0.0)
                sb_calls = [cl for cl in calls if cl[0] == si] if do_gather else []
                mm_bank_seq = []            # bank of each MM in emission order
                for (_, _, run) in sb_calls:
                    for (_, bi, _) in run:
                        mm_bank_seq.append((bi * 128) // 512)
                first_of_bank, last_of_bank = {}, {}
                for i, bk in enumerate(mm_bank_seq):
                    if bk not in first_of_bank:
                        first_of_bank[bk] = i
                    last_of_bank[bk] = i
                mm_i = 0
                for (csi, k, run) in sb_calls:
                    nt = len(run)
                    tau0 = run[0][0]
                    gst = wp.tile([128, MAXNT * 128], BF16, tag="gst", bufs=3)
                    if nt * 128 not in nidx_regs:
                        nidx_regs[nt * 128] = nc.gpsimd.to_reg(nt * 128)
                    nc.gpsimd.dma_gather(
                        out_ap=gst[:, 0:nt * 128].rearrange("p (t e) -> p t e", t=nt),
                        in_ap=ztab[k * CH:(k + 1) * CH, :],
                        idxs_ap=idx_all[:, tau0 * 8:(tau0 + nt) * 8],
                        num_idxs=nt * 128,
                        num_idxs_reg=nidx_regs[nt * 128],
                        elem_size=D,
                        single_packet=(nt * 128 <= 1024))
                    # build ALL S tiles of the run in one DVE op:
                    # S[p, t, e] = (iota[p, e] == reld[p, tau0+t])
                    S = wp.tile([128, MAXNT * 128], BF16, tag="S", bufs=3)
                    iap = iota_t[:]
                    rap = reld_t[:, tau0:tau0 + nt]
                    nc.vector.tensor_tensor(
                        out=S[:, 0:nt * 128].rearrange("p (t e) -> p t e", t=nt),
                        in0=_bcast_ap(iap, [iap.ap[0], [0, nt], [1, 128]]),
                        in1=_bcast_ap(rap, [rap.ap[0], [1, nt], [0, 128]]),
                        op=ALU.is_equal)
                    for ti, (tau, bi, b) in enumerate(run):
                        bk = mm_bank_seq[mm_i]
                        nc.tensor.matmul(out=agg[:, bi * 128:(bi + 1) * 128],
                                         lhsT=S[:, ti * 128:(ti + 1) * 128],
                                         rhs=gst[:, ti * 128:(ti + 1) * 128],
                                         start=(first_of_bank[bk] == mm_i),
                                         stop=(last_of_bank[bk] == mm_i),
                                         skip_group_check=True)
                        mm_i += 1

                # finalize:  out = LN2( (rate*(cnt*z + aggn + cnt*bfc) + gamma)
                #                       / (1 + rate*deg + EPS) - z )
                # where z = zn + bfc, and cnt*z + aggn + cnt*bfc = cnt*(zn+bfc) + aggn
                for bi, b in enumerate(sb):
                    sl = slice(bi * 128, (bi + 1) * 128)
                    u = wp.tile([128, 128], F32, tag="fin_u")
                    nc.vector.tensor_add(out=u[:], in0=zn_sb[:, sl], in1=bfc_t)
                    t1 = wp.tile([128, 128], F32, tag="fin1")
                    nc.vector.scalar_tensor_tensor(
                        out=t1[:], in0=u[:], scalar=cnt_t[:, b:b + 1],
                        in1=agg[:, sl], op0=ALU.mult, op1=ALU.add)
                    num = wp.tile([128, 128], F32, tag="fin2")
                    nc.vector.tensor_tensor(out=num[:], in0=rate_sb[:, sl],
                                            in1=t1[:], op=ALU.mult)
                    nc.vector.tensor_add(out=num[:], in0=num[:], in1=gam_sb[:, sl])
                    den = wp.tile([128, 128], F32, tag="fin3")
                    nc.vector.tensor_scalar(out=den[:], in0=rate_sb[:, sl],
                                            scalar1=deg_t[:, b:b + 1],
                                            scalar2=1.0 + EPS,
                                            op0=ALU.mult, op1=ALU.add)
                    nc.scalar.activation(out=den[:], in_=den[:], func=AF.Ln)
                    nc.scalar.activation(out=den[:], in_=den[:], func=AF.Exp,
                                         scale=-1.0)
                    nc.vector.tensor_tensor(out=num[:], in0=num[:], in1=den[:],
                                            op=ALU.mult)
                    nc.vector.tensor_tensor(out=num[:], in0=num[:], in1=u[:],
                                            op=ALU.subtract)
                    _emit_ln(nc, wp, num[:], ln2g_t, ln2b_t, out_sb[:, sl], "ln2",
                             D, eps_t)
                nc.sync.dma_start(
                    out=out_d[si * SBS * 128:si * SBS * 128 + nsb * 128, :].rearrange(
                        "(p j) c -> p (j c)", p=128),
                    in_=out_sb[:, 0:nsb * 128])

    return nc


def postprocess_for_hw(nc):
    """Must run after build_bass and before NEFF compile (not before CoreSim)."""
    patch_library_reloads(nc)
    split_sync_waits(nc, max_waits=1)


# ----------------------------------------------------------------- host prep

def prepare_core_inputs(x, degree, W_fc, b_fc, W_rate, W1, b1, W2, b2,
                        ln1_g, ln1_b, ln2_g, ln2_b, sch, n_cores, D=128, DH=256):
    """Returns (shared_inputs dict, per_core list of dicts)."""
    N = x.shape[0]
    NS, NB, NPAD2 = sch["NS"], sch["NB"], sch["NPAD2"]
    NSP = NB * 128
    NG = NPAD2 // 1024
    import ml_dtypes
    xt_full = np.zeros((D, NPAD2), dtype=ml_dtypes.bfloat16)
    xt_full[:, :N] = x.T.astype(ml_dtypes.bfloat16)
    # permute group-internal node order: column g*1024 + j*128 + p holds
    # node g*1024 + p*8 + j
    xt_full = np.ascontiguousarray(
        xt_full.reshape(D, NG, 128, 8).transpose(0, 1, 3, 2).reshape(D, NPAD2))
    wcat = np.concatenate([W_fc.T, W_rate.T], axis=1).astype(np.float32)
    w1t = np.ascontiguousarray(W1.T.astype(np.float32))
    w2t = np.ascontiguousarray(W2.T.astype(np.float32))
    b1p = b1.astype(np.float32).reshape(2, 128).T    # [128, 2]
    consts = np.concatenate([
        np.tile(b_fc.astype(np.float32), (128, 1)),
        np.tile(b2.astype(np.float32), (128, 1)),
        np.tile(ln1_g.astype(np.float32), (128, 1)),
        np.tile(ln1_b.astype(np.float32), (128, 1)),
        np.tile(ln2_g.astype(np.float32), (128, 1)),
        np.tile(ln2_b.astype(np.float32), (128, 1)),
        np.full((128, 1), LN_EPS, dtype=np.float32),
        np.full((128, 1), 1.0, dtype=np.float32),
        b1p,
    ], axis=1)
    iotab = np.tile(np.arange(128, dtype=np.float32), (128, 1)).astype(
        ml_dtypes.bfloat16)
    wfcb = np.ascontiguousarray(W_fc.T.astype(ml_dtypes.bfloat16))
    shared = dict(xt_full=xt_full, wcat=wcat, w1t=w1t, w2t=w2t,
                  consts=np.ascontiguousarray(consts), iotab=iotab, wfcb=wfcb)

    cnt = sch["cnt"]
    per_core = []
    for c in range(n_cores):
        xt_loc = np.zeros((D, NSP), dtype=np.float32)
        xt_loc[:, :NS] = x[c * NS:(c + 1) * NS].T
        cntp = np.zeros((128, NB), dtype=np.float32)
        degp = np.zeros((128, NB), dtype=np.float32)
        cseg = np.zeros(NSP, dtype=np.float32)
        cseg[:NS] = cnt[c * NS:(c + 1) * NS]
        dseg = np.zeros(NSP, dtype=np.float32)
        dseg[:NS] = degree[c * NS:(c + 1) * NS]
        cntp[:, :] = cseg.reshape(NB, 128).T
        degp[:, :] = dseg.reshape(NB, 128).T
        per_core.append(dict(xt_loc=np.ascontiguousarray(xt_loc),
                             idxp=sch["idxp"][c], reld=sch["reld"][c],
                             cntp=np.ascontiguousarray(cntp),
                             degp=np.ascontiguousarray(degp)))
    return shared, per_core


def unpermute_out(dev_out, sch):
    """Undo the device's per-superblock row permutation: device row
    si*1024 + p*nsb + j  holds node  si*1024 + j*128 + p."""
    NB, SBS = sch["NB"], sch["SBS"]
    NSP = NB * 128
    nat = np.empty_like(dev_out)
    for si, sb in enumerate(sch["sbs"]):
        nsb = len(sb)
        r0 = si * SBS * 128
        blk = dev_out[r0:r0 + nsb * 128]
        nat[r0:r0 + nsb * 128] = (
            blk.reshape(128, nsb, -1).transpose(1, 0, 2).reshape(nsb * 128, -1))
    return nat


# ----------------------------------------------------------------- numpy ref

def numpy_reference(x, edge_index, degree, W_fc, b_fc, W_rate, W1, b1, W2, b2,
                    ln1_g, ln1_b, ln2_g, ln2_b):
    def ln(v, g, b):
        m = v.mean(-1, keepdims=True)
        var = ((v - m) ** 2).mean(-1, keepdims=True)
        return (v - m) / np.sqrt(var + LN_EPS) * g + b

    def softplus(v):
        return np.log1p(np.exp(-np.abs(v))) + np.maximum(v, 0)

    rate = softplus(x @ W_rate.T)
    h = softplus(x @ W1.T + b1)
    gamma = ln(h @ W2.T + b2, ln1_g, ln1_b)
    z = x @ W_fc.T + b_fc
    row, col = edge_index[0], edge_index[1]
    msg = z[row] + z[col]
    agg = np.zeros_like(z)
    np.add.at(agg, row, msg)
    out = (rate * agg + gamma) / (1.0 + rate * degree[:, None] + EPS) - z
    return ln(out, ln2_g, ln2_b)


# ----------------------------------------------------------------- runner

N_CORES = 8
_SHARED_NAMES = ("xt_full", "wcat", "w1t", "w2t", "consts", "iotab", "wfcb")


def make_runner(nc, shared, per_core, n_cores=N_CORES):
    """Compile nc via PJRT/axon and return (fn, dev_args, out_info).

    Shared inputs are replicated (one upload), per-core inputs sharded on
    axis 0. Output buffers are passed as (unread) operands so repeated calls
    need no fresh allocations. Call fn(*dev_args) -> tuple of out arrays.
    """
    import jax
    from jax.sharding import Mesh, PartitionSpec, NamedSharding
    from jax.experimental.shard_map import shard_map
    from concourse import bass2jax

    bass2jax.install_neuronx_cc_hook()

    in_names, out_names, out_avals, zero_outs = [], [], [], []
    partition_name = (nc.partition_id_tensor.name
                      if nc.partition_id_tensor else None)
    for alloc in nc.m.functions[0].allocations:
        if not isinstance(alloc, mybir.MemoryLocationSet):
            continue
        name = alloc.memorylocations[0].name
        if alloc.kind == "ExternalInput":
            if name != partition_name:
                in_names.append(name)
        elif alloc.kind == "ExternalOutput":
            shape = tuple(alloc.tensor_shape)
            dtype = mybir.dt.np(alloc.dtype)
            out_names.append(name)
            out_avals.append(jax.core.ShapedArray(shape, dtype))
            zero_outs.append(np.zeros(shape, dtype))
    n_params = len(in_names)
    all_in = list(in_names) + list(out_names)
    if partition_name is not None:
        all_in.append(partition_name)

    def _body(*args):
        operands = list(args)
        if partition_name is not None:
            operands.append(bass2jax.partition_id_tensor())
        outs = bass2jax._bass_exec_p.bind(
            *operands,
            out_avals=tuple(out_avals),
            in_names=tuple(all_in),
            out_names=tuple(out_names),
            lowering_input_output_aliases=(),
            sim_require_finite=True,
            sim_require_nnan=True,
            nc=nc)
        return tuple(outs)

    devices = jax.devices()[:n_cores]
    mesh = Mesh(np.asarray(devices), ("core",))
    specs = []
    host_args = []
    for name in in_names:
        if name in _SHARED_NAMES:
            specs.append(PartitionSpec())
            host_args.append(np.asarray(shared[name]))
        else:
            specs.append(PartitionSpec("core"))
            host_args.append(np.concatenate(
                [np.asarray(per_core[c][name]) for c in range(n_cores)], axis=0))
    for z in zero_outs:
        specs.append(PartitionSpec("core"))
        host_args.append(np.zeros((n_cores * z.shape[0], *z.shape[1:]), z.dtype))
    out_specs = (PartitionSpec("core"),) * len(out_names)

    def _chain(rep):
        def body(*args):
            ins = list(args[:n_params])
            outs = tuple(args[n_params:])
            for _ in range(rep):
                outs = _body(*ins, *outs)
            return outs
        return body

    fn = jax.jit(shard_map(_chain(1), mesh=mesh, in_specs=tuple(specs),
                           out_specs=out_specs, check_rep=False),
                 keep_unused=True)
    fn_rep = None  # multi-call chains unsupported by neuronx_cc_hook
    dev_args = [jax.device_put(a, NamedSharding(mesh, s))
                for a, s in zip(host_args, specs)]
    out_shapes = [tuple(a.shape) for a in out_avals]
    return fn, dev_args, (out_names, out_shapes, fn_rep)


def _prepare_all(inputs, n_cores=N_CORES):
    x = np.asarray(inputs["x"], dtype=np.float32)
    N = x.shape[0]
    sch = build_schedule(np.asarray(inputs["edge_index"]), N, n_cores)
    nc = build_bass(sch, n_cores)
    postprocess_for_hw(nc)
    shared, per_core = prepare_core_inputs(
        x, np.asarray(inputs["degree"], dtype=np.float32),
        np.asarray(inputs["W_fc"]), np.asarray(inputs["b_fc"]),
        np.asarray(inputs["W_rate"]), np.asarray(inputs["W1"]),
        np.asarray(inputs["b1"]), np.asarray(inputs["W2"]),
        np.asarray(inputs["b2"]), np.asarray(inputs["ln1_g"]),
        np.asarray(inputs["ln1_b"]), np.asarray(inputs["ln2_g"]),
        np.asarray(inputs["ln2_b"]), sch, n_cores)
    return sch, nc, shared, per_core


def run_kernel(inputs, n_cores=N_CORES, time_reps=0):
    """Returns (out [N, D] fp32, exec_ns or None)."""
    import jax, time as _time
    sch, nc, shared, per_core = _prepare_all(inputs, n_cores)
    fn, dev_args, (out_names, out_shapes, fn_rep) = make_runner(
        nc, shared, per_core, n_cores)
    outs = fn(*dev_args)
    jax.block_until_ready(outs)
    exec_ns = None
    if time_reps:
        # wall time of a dispatch; dominated by ~30-75 ms axon RPC overhead,
        # so this is an upper bound on device time.
        ts = []
        for _ in range(time_reps):
            t0 = _time.perf_counter()
            o1 = fn(*dev_args); jax.block_until_ready(o1)
            ts.append(_time.perf_counter() - t0)
        exec_ns = int(min(ts) * 1e9)
    oi = out_names.index("out")
    N = np.asarray(inputs["x"]).shape[0]
    NS, rows = sch["NS"], out_shapes[oi][0]
    full = np.asarray(outs[oi]).reshape(n_cores, rows, -1)
    out = np.concatenate(
        [unpermute_out(full[c], sch)[:NS] for c in range(n_cores)], axis=0)[:N]
    return np.ascontiguousarray(out.astype(np.float32)), exec_ns


def kernel(**inputs):
    out, _ = run_kernel(inputs)
    return out
